# revision 46
# baseline (speedup 1.0000x reference)
"""MeshMeanFlowNet block on 8 Trainium2 NeuronCores.

Sharding: data-parallel over B (one batch element per core), no collectives.

Key design points vs the naive formulation:
- All activations feature-major ([feature, token]); attention softmax in the
  transposed layout S^T[j, i] (key j on partitions) so the PV matmul consumes
  probabilities directly as the moving operand and the softmax denominator
  falls out of a ones-row appended to V.
- The per-edge-type/per-head bias is applied MULTIPLICATIVELY after exp:
  P = exp(S) * g, with g[h][j,i] = exp(edge_table[edge[i,j], h]) precomputed
  on the host and streamed from HBM as bf16 tiles. This removes every
  mask/select elementwise op from the device inner loop (1 Act exp + 1 DVE
  bf16 multiply per (key-tile, head) pair).
- LayerNorm affine is applied as h = x*R1 + Dt where R1 = s1 (x) r and
  Dt = t1 (x) 1 - s1 (x) (m*r) are built by rank-1/rank-2 PE matmuls into
  PSUM (2 DVE passes per 128-feature tile, no gpsimd broadcasts, no
  column-transpose DMA storms).
- Softmax normalization: Z rows extracted from PSUM, packed into an [8, V]
  tile via tiny SBUF->SBUF DMAs, one reciprocal, then per-feature-tile
  band-broadcast via contraction-2 PE matmuls; one in-place DVE multiply
  per attention-output tile.
- Weights are shipped bf16 (halves weight HBM traffic); f32r only where
  fp32-ish accumulation inputs matter (x, LN sums).
"""

import sys

sys.path.insert(0, "/opt/trn_rl_repo")

import ml_dtypes
import numpy as np

B, V, D, H = 8, 1024, 512, 8
HD = D // H  # 64
NCORES = 8

_cache = {}


def _build_program(sim_mode=False):
    import contextlib

    import concourse.bacc as bacc
    import concourse.tile as tile
    from concourse import mybir

    f32 = mybir.dt.float32
    f32r = mybir.dt.float32r
    bf16 = mybir.dt.bfloat16
    f8 = mybir.dt.float8e4
    DR = mybir.MatmulPerfMode.DoubleRow
    ALU = mybir.AluOpType
    ACTF = mybir.ActivationFunctionType

    nc = bacc.Bacc("TRN2", target_bir_lowering=False, debug=False,
                   num_devices=NCORES)

    # ---- DRAM I/O (per-core shard, host pre-laid-out) ----
    xT = nc.dram_tensor("xT", [D, V], f32r, kind="ExternalInput")
    condc = nc.dram_tensor("condc", [4, 128], f32, kind="ExternalInput")
    gbias = nc.dram_tensor("gbias", [H * V, V], bf16, kind="ExternalInput")
    wqk = nc.dram_tensor("wqk", [D, 1024], bf16, kind="ExternalInput")
    wv = nc.dram_tensor("wv", [D, 512], bf16, kind="ExternalInput")
    wada = nc.dram_tensor("wada", [D, 2048], bf16, kind="ExternalInput")
    badar = nc.dram_tensor("badar", [1, 2048], f32, kind="ExternalInput")
    wproj = nc.dram_tensor("wproj", [D, D], bf16, kind="ExternalInput")
    bproj = nc.dram_tensor("bproj", [4, 128], f32, kind="ExternalInput")
    wm1 = nc.dram_tensor("wm1", [D, 2048], bf16, kind="ExternalInput")
    bm1 = nc.dram_tensor("bm1", [16, 128], f32, kind="ExternalInput")
    wm2 = nc.dram_tensor("wm2", [2048, D], bf16, kind="ExternalInput")
    bm2 = nc.dram_tensor("bm2", [4, 128], f32, kind="ExternalInput")
    bandd = nc.dram_tensor("bandd", [2, 128], bf16, kind="ExternalInput")
    onesd = nc.dram_tensor("onesd", [128, 1], f32r, kind="ExternalInput")
    onesrowd = nc.dram_tensor("onesrowd", [1, 1024], f32r,
                              kind="ExternalInput")
    yT = nc.dram_tensor("yT", [D, V], f32, kind="ExternalOutput")

    def mm(out, lhsT, rhs, **kw):
        nc.tensor.matmul(out, lhsT, rhs, **kw)

    with tile.TileContext(nc) as tc:
        with contextlib.ExitStack() as ctx:
            persist = ctx.enter_context(tc.tile_pool(name="persist", bufs=1))

            ones = persist.tile([128, 1], f32r, tag="ones")
            nc.sync.dma_start(out=ones, in_=onesd[:])
            onesrow = persist.tile([1, V], f32r, tag="onesrow")
            nc.sync.dma_start(out=onesrow, in_=onesrowd[:])
            epst = persist.tile([1, 1], f32, tag="eps")
            nc.vector.memset(epst, 1e-5)
            # band indicator for Z-broadcast: row0 -> out partitions 0-63,
            # row1 -> 64-127
            bandm = persist.tile([2, 128], bf16, tag="bandm")
            nc.sync.dma_start(out=bandm, in_=bandd[:])

            # x (feature-major, also becomes x2 in place after the residual)
            xT_t = [persist.tile([128, V], f32r, tag=f"xT{kc}",
                                 name=f"xT_t{kc}") for kc in range(4)]

            # row-form AdaLN params: [1, 2048] =
            # [ada1: scale(512) shift(512) | ada2: scale shift] (+1 folded
            # into scale on host)
            rows_params = persist.tile([1, 2048], f32r, tag="rparams")

            # LN1 sums + param path share one startup window: wada DMAs
            # interleave with x so the PE never idles >3.4us (HAM throttle);
            # ln1t is pushed before adaw so adaw can close in stack order
            ln1t = ctx.enter_context(tc.tile_pool(name="ln1t", bufs=1))
            with tc.tile_pool(name="adaw", bufs=1) as adaw:
                condt = adaw.tile([128, 5], f32, tag="cond")
                nc.sync.dma_start(out=condt[:, 0:4],
                                  in_=condc[:].rearrange("c p -> p c"))
                badar_t = adaw.tile([1, 2048], f32, tag="badar")
                nc.sync.dma_start(out=badar_t, in_=badar[:])
                wada_t = [adaw.tile([128, 2048], bf16, tag=f"wada{kc}",
                                    name="wada_t") for kc in range(4)]
                for kc in range(4):
                    for nh in range(2):
                        s = slice(nh * 512, nh * 512 + 512)
                        nc.sync.dma_start(out=xT_t[kc][:, s],
                                          in_=xT[kc * 128:(kc + 1) * 128, s])
                    nc.sync.dma_start(out=wada_t[kc],
                                      in_=wada[kc * 128:(kc + 1) * 128, :])
                bp_t = persist.tile([128, 4], f32, tag="bproj")
                nc.sync.dma_start(out=bp_t,
                                  in_=bproj[:].rearrange("c p -> p c"))
                bm1_t = persist.tile([128, 16], f32, tag="bm1")
                nc.sync.dma_start(out=bm1_t,
                                  in_=bm1[:].rearrange("c p -> p c"))
                bm2_t = persist.tile([128, 4], f32, tag="bm2")
                nc.sync.dma_start(out=bm2_t,
                                  in_=bm2[:].rearrange("c p -> p c"))
                lnp0_cm = tc.tile_pool(name="lnp0", bufs=1, space="PSUM")
                lnp0 = lnp0_cm.__enter__()
                ln1sums = ln_sums(xT_t, ln1t, lnp0)
                with tc.tile_pool(name="adap", bufs=1,
                                  space="PSUM") as adap:
                    nc.vector.memset(condt[:, 4:5], 0.0)
                    scond = adaw.tile([128, 5], bf16, tag="scond")
                    if sim_mode:
                        sig = adaw.tile([128, 5], f32, tag="sig")
                        nc.scalar.activation(sig, condt, ACTF.Sigmoid)
                        nc.vector.tensor_mul(scond, condt, sig)
                    else:
                        nc.scalar.activation(scond, condt, ACTF.Silu)
                    pp = adap.tile([2, 2048], f32, tag="pada")
                    for kc in range(4):
                        for oc in range(4):
                            s = slice(oc * 512, oc * 512 + 512)
                            mm(pp[:, s], scond[:, kc:kc + 2],
                               wada_t[kc][:, s],
                               start=(kc == 0), stop=(kc == 3))
                    nc.vector.tensor_add(rows_params, pp[0:1, :], badar_t)

            def ln_sums(src_tiles, lnt, lnp):
                """Phase 1 of adaln: squares + ones-matmul sums (only needs
                src tiles, no params)."""
                ps_s = lnp.tile([1, V], f32, tag="lnsum")
                ps_q = lnp.tile([1, V], f32, tag="lnsqsum")
                for kc in range(4):
                    sq = lnt.tile([128, V], f32r, tag="lnsq", bufs=2,
                                  name="sq")
                    nc.scalar.square(sq, src_tiles[kc].bitcast(f32))
                    for nh in range(2):
                        s = slice(nh * 512, nh * 512 + 512)
                        mm(ps_s[:, s], ones, src_tiles[kc][:, s],
                           start=(kc == 0), stop=(kc == 3))
                        mm(ps_q[:, s], ones, sq[:, s],
                           start=(kc == 0), stop=(kc == 3))
                return ps_s, ps_q

            def adaln(src_tiles, ln_idx, dst_pool, out_tag,
                      pair_fp8=False, presums=None, prelnt=None):
                """Feature-axis LayerNorm + adaptive affine. Returns 4
                feature-major bf16 tiles, or (pair_fp8) 2 DoubleRow-paired
                fp8 tiles [128, 2, V]."""
                if pair_fp8:
                    out = [dst_pool.tile([128, 2, V], f8,
                                         tag=f"{out_tag}{kcp}",
                                         name=f"ln_{out_tag}{kcp}")
                           for kcp in range(2)]
                else:
                    out = [dst_pool.tile([128, V], bf16,
                                         tag=f"{out_tag}{kc}",
                                         name=f"ln_{out_tag}{kc}")
                           for kc in range(4)]
                base = ln_idx * 1024
                import contextlib as _ctx
                with _ctx.ExitStack() as lstk:
                    if prelnt is None:
                        lnt = lstk.enter_context(
                            tc.tile_pool(name="lnt", bufs=1))
                    else:
                        lnt = prelnt
                    with _ctx.ExitStack() as pstk:
                        if presums is None:
                            lnp = pstk.enter_context(
                                tc.tile_pool(name="lnp", bufs=1,
                                             space="PSUM"))
                            ps_s, ps_q = ln_sums(src_tiles, lnt, lnp)
                        else:
                            ps_s, ps_q = presums
                        m_row = lnt.tile([1, V], f32, tag="mrow")
                        nc.scalar.mul(m_row, ps_s, 1.0 / D)
                        msq = lnt.tile([1, V], f32, tag="msq")
                        nc.scalar.square(msq, m_row)
                        varr = lnt.tile([1, V], f32, tag="varr")
                        nc.vector.scalar_tensor_tensor(
                            varr, ps_q, 1.0 / D, msq, ALU.mult, ALU.subtract)
                        if presums is not None:
                            lnp0.close()
                    stdr = lnt.tile([1, V], f32, tag="stdr")
                    nc.scalar.activation(stdr, varr, ACTF.Sqrt, bias=epst)
                    with nc.allow_low_precision(
                            reason="1/std via fast approx"):
                        nc.vector.reciprocal_approx_fast(out=stdr, in_=stdr)
                    r_row = lnt.tile([1, V], f32r, tag="rrow")
                    nc.vector.tensor_copy(out=r_row, in_=stdr)
                    mrneg = lnt.tile([1, V], f32r, tag="mrneg")
                    nc.vector.scalar_tensor_tensor(
                        mrneg, m_row, -1.0, stdr,
                        ALU.mult, ALU.mult)
                    with tc.tile_pool(name="lnbp", bufs=2,
                                      space="PSUM") as lnbp:
                        for kc in range(4):
                            s1r = rows_params[0:1, base + kc * 128:
                                              base + kc * 128 + 128]
                            t1r = rows_params[0:1, base + 512 + kc * 128:
                                              base + 512 + kc * 128 + 128]
                            R1 = lnbp.tile([128, V], f32, tag="R1",
                                           name="R1")
                            Dt = lnbp.tile([128, V], f32, tag="Dt",
                                           name="Dt")
                            for nh in range(2):
                                s = slice(nh * 512, nh * 512 + 512)
                                mm(R1[:, s], s1r,
                                   r_row[:, s], start=True, stop=True)
                                mm(Dt[:, s], t1r,
                                   onesrow[:, s], start=True, stop=False)
                                mm(Dt[:, s], s1r,
                                   mrneg[:, s], start=False, stop=True)
                            u = lnt.tile([128, V], f32, tag="lnu", bufs=2,
                                         name="u")
                            nc.vector.tensor_mul(u, src_tiles[kc].bitcast(f32),
                                                 R1)
                            dst = (out[kc // 2][:, kc % 2, :] if pair_fp8
                                   else out[kc])
                            nc.vector.tensor_add(dst, u, Dt)
                return out

            # proj+mlp weights: pool outlives the attention scope; DMAs are
            # issued inside the attention loop so they stream during it
            mlpw = ctx.enter_context(tc.tile_pool(name="mlpw", bufs=1))
            wp_t = [mlpw.tile([128, 512], bf16, tag=f"wproj{kc}",
                              name="wp_t") for kc in range(4)]
            wm1_t = [mlpw.tile([128, 2048], bf16, tag=f"wm1{kc}",
                               name="wm1_t") for kc in range(4)]
            wm2_t = [mlpw.tile([128, 512], bf16, tag=f"wm2{kc}",
                               name="wm2_t") for kc in range(16)]

            # qk tiles (feature-major q then k), token-major v (+ones row)
            with tc.tile_pool(name="attlife", bufs=1) as attlife:
                qk = [attlife.tile([128, V], bf16, tag=f"qk{m}",
                                   name=f"qk{m}") for m in range(8)]
                vaug = [attlife.tile([128, 8, 65], bf16, tag=f"vaug{t}",
                                     name=f"vaug{t}") for t in range(8)]
                att = [attlife.tile([128, V], bf16, tag=f"att{kc}",
                                    name=f"att{kc}") for kc in range(4)]
                # Z rows packed [parity, kc*V + i]: head h -> row h%2,
                # free slice (h//2)*V
                zcat = attlife.tile([2, 4 * V], f32, tag="zcat")

                # h1 = AdaLN1(x); qk feature-major; v token-major
                with tc.tile_pool(name="h1pool", bufs=1) as h1pool:
                    h1 = adaln(xT_t, 0, h1pool, "h1",
                               presums=ln1sums, prelnt=ln1t)
                    with tc.tile_pool(name="qkvw", bufs=1) as qkvw, \
                            tc.tile_pool(name="qkp", bufs=3,
                                         space="PSUM") as qkp, \
                            tc.tile_pool(name="qkvp", bufs=2,
                                         space="PSUM") as qkvp:
                        wqk_t = [qkvw.tile([128, 1024], bf16, tag=f"wqk{kc}",
                                           name="wqk_t") for kc in range(4)]
                        wv_t = [qkvw.tile([128, 512], bf16, tag=f"wv{kc}",
                                          name="wv_t") for kc in range(4)]
                        for kc in range(4):
                            nc.sync.dma_start(
                                out=wqk_t[kc],
                                in_=wqk[kc * 128:(kc + 1) * 128, :])
                            nc.sync.dma_start(
                                out=wv_t[kc],
                                in_=wv[kc * 128:(kc + 1) * 128, :])
                        # emit (q, k) tile pairs per head-pair so attention
                        # for early heads can start before qkv finishes;
                        # one wide psum + single evac per m halves the DVE
                        # copy backlog the attention P-muls queue behind
                        for m in (0, 4, 1, 5, 2, 6, 3, 7):
                            pp = qkp.tile([128, V], f32, tag="mmqk")
                            for nh in range(2):
                                s = slice(nh * 512, nh * 512 + 512)
                                for kc in range(4):
                                    mm(pp[:, s],
                                       wqk_t[kc][:, m * 128:(m + 1) * 128],
                                       h1[kc][:, s], start=(kc == 0),
                                       stop=(kc == 3))
                            nc.vector.tensor_copy(out=qk[m], in_=pp)
                        for t in range(8):
                            pp = qkvp.tile([128, 512], f32, tag="mmv")
                            for kc in range(4):
                                mm(pp, h1[kc][:, t * 128:(t + 1) * 128],
                                   wv_t[kc], start=(kc == 0), stop=(kc == 3))
                            nc.vector.tensor_copy(
                                out=vaug[t][:, :, 0:64],
                                in_=pp[:].rearrange("p (h d) -> p h d", h=8))
                            nc.gpsimd.memset(vaug[t][:, :, 64:65], 1.0)

                # attention: S^T[j,i]; P = exp(S) * g; softmax denom from
                # the vaug ones-row, normalized after the loop. Software-
                # pipelined with LA tiles of lookahead so the PE never
                # stalls on the exp->mul chain of the pair it just scored.
                LA = 3
                with tc.tile_pool(name="attt", bufs=1) as attt:
                  with tc.tile_pool(name="attps", bufs=LA,
                                    space="PSUM") as attps, \
                        tc.tile_pool(name="attpo", bufs=1,
                                     space="PSUM") as attpo:
                    for h in range(8):
                        ops = attpo.tile([65, V], f32, tag="ops",
                                         bufs=1, name="ops")
                        Ss = {}
                        gs = {}

                        def emit(h, jt):
                            g_t = attt.tile([128, V], bf16, tag="gt",
                                            bufs=12, name="g_t")
                            nc.sync.dma_start(
                                out=g_t,
                                in_=gbias[(h * 8 + jt) * 128:
                                          (h * 8 + jt) * 128 + 128, :])
                            gs[jt] = g_t
                            kt = qk[4 + h // 2][
                                (h % 2) * 64:(h % 2) * 64 + 64,
                                jt * 128:jt * 128 + 128]
                            S = attps.tile([128, V], f32, tag="mms",
                                           name="S")
                            for nh in range(2):
                                s = slice(nh * 512, nh * 512 + 512)
                                qt = qk[h // 2][
                                    (h % 2) * 64:(h % 2) * 64 + 64, s]
                                mm(S[:, s], kt, qt, start=True, stop=True)
                            Ss[jt] = S

                        for jt in range(LA):
                            emit(h, jt)
                        for jt in range(8):
                            S = Ss.pop(jt)
                            g_t = gs.pop(jt)
                            P0 = attt.tile([128, V], bf16, tag="P0",
                                           bufs=3, name="P0")
                            nc.scalar.activation(P0, S, ACTF.Exp)
                            P = attt.tile([128, V], bf16, tag="P",
                                          bufs=3, name="P")
                            nc.vector.tensor_mul(P, P0, g_t)
                            for nh in range(2):
                                s = slice(nh * 512, nh * 512 + 512)
                                mm(ops[:, s], vaug[jt][:, h, :],
                                   P[:, s], start=(jt == 0),
                                   stop=(jt == 7))
                            if jt + LA < 8:
                                emit(h, jt + LA)
                            if h == 4 and jt == 0:
                                for kc in range(4):
                                    nc.sync.dma_start(
                                        out=wp_t[kc],
                                        in_=wproj[kc * 128:(kc + 1) * 128, :])
                            if h == 5 and jt == 0:
                                for kc in range(4):
                                    nc.sync.dma_start(
                                        out=wm1_t[kc],
                                        in_=wm1[kc * 128:(kc + 1) * 128, :])
                            if h == 6 and jt == 0:
                                for kc in range(16):
                                    nc.sync.dma_start(
                                        out=wm2_t[kc],
                                        in_=wm2[kc * 128:(kc + 1) * 128, :])
                        ztmp = attt.tile([65, V], f32, tag="ztmp",
                                         bufs=2, name="ztmp")
                        nc.vector.tensor_copy(out=ztmp[64:65, :],
                                              in_=ops[64:65, :])
                        nc.sync.dma_start(
                            out=zcat[h % 2:h % 2 + 1,
                                     (h // 2) * V:(h // 2) * V + V],
                            in_=ztmp[64:65, :])
                        nc.vector.tensor_copy(
                            out=att[h // 2][(h % 2) * 64:
                                            (h % 2) * 64 + 64, :],
                            in_=ops[0:64, :])


                  # normalize: att[kc] rows 0-63 = head 2kc, 64-127 = 2kc+1
                  if True:
                    with nc.allow_low_precision(
                            reason="softmax denom reciprocal"):
                        nc.vector.reciprocal_approx_fast(out=zcat, in_=zcat)
                    rinv = attt.tile([2, 4 * V], bf16, tag="rinv")
                    nc.vector.tensor_copy(out=rinv, in_=zcat)
                    with tc.tile_pool(name="zbp", bufs=2,
                                      space="PSUM") as zbp:
                        for kc in range(4):
                            zb = zbp.tile([128, V], f32, tag="zb", name="zb")
                            for nh in range(2):
                                s = slice(nh * 512, nh * 512 + 512)
                                mm(zb[:, s], bandm,
                                   rinv[0:2, kc * V + nh * 512:
                                        kc * V + nh * 512 + 512],
                                   start=True, stop=True)
                            nc.vector.tensor_mul(att[kc], att[kc], zb)

                # proj + residual (in place into xT_t -> x2)
                with tc.tile_pool(name="projp", bufs=4,
                                  space="PSUM") as projp:
                    for m in range(4):
                        for nh in range(2):
                            s = slice(nh * 512, nh * 512 + 512)
                            pp = projp.tile([128, 512], f32, tag="mmproj")
                            for kc in range(4):
                                mm(pp, wp_t[kc][:, m * 128:(m + 1) * 128],
                                   att[kc][:, s], start=(kc == 0),
                                   stop=(kc == 3))
                            nc.vector.scalar_tensor_tensor(
                                xT_t[m][:, s], pp,
                                bp_t[:, m:m + 1],
                                xT_t[m][:, s].bitcast(f32), ALU.add,
                                ALU.add)

            # ---------- MLP branch (xT_t now holds x2) ----------
            with tc.tile_pool(name="mlplife", bufs=1) as mlplife:
                h2 = adaln(xT_t, 1, mlplife, "h2")
                with tc.tile_pool(name="mlpt", bufs=1) as mlpt, \
                        tc.tile_pool(name="mlpp", bufs=4,
                                     space="PSUM") as mlpp:
                    for nh in range(2):
                        s = slice(nh * 512, nh * 512 + 512)
                        gm = [mlpt.tile([128, 512], bf16, tag=f"gm{m}",
                                        name=f"gm{m}") for m in range(16)]
                        for m in range(16):
                            pp = mlpp.tile([128, 512], f32, tag="mmm1")
                            for kc in range(4):
                                mm(pp, wm1_t[kc][:, m * 128:(m + 1) * 128],
                                   h2[kc][:, s], start=(kc == 0),
                                   stop=(kc == 3))
                            if sim_mode:
                                sig = mlpt.tile([128, 512], f32, tag="gsig",
                                                bufs=2, name="gsig")
                                nc.scalar.activation(sig, pp, ACTF.Sigmoid,
                                                     scale=1.702)
                                nc.vector.tensor_mul(gm[m], pp, sig)
                            else:
                                nc.scalar.activation(gm[m], pp, ACTF.Gelu,
                                                     bias=bm1_t[:, m:m + 1])
                        for m in range(4):
                            pp = mlpp.tile([128, 512], f32, tag="mmm2")
                            for kc in range(16):
                                mm(pp, wm2_t[kc][:, m * 128:(m + 1) * 128],
                                   gm[kc], start=(kc == 0), stop=(kc == 15))
                            yt = mlpt.tile([128, 512], f32, tag="yt",
                                           bufs=2, name="yt")
                            nc.vector.scalar_tensor_tensor(
                                yt, pp, bm2_t[:, m:m + 1],
                                xT_t[m][:, s].bitcast(f32), ALU.add,
                                ALU.add)
                            nc.sync.dma_start(
                                out=yT[m * 128:(m + 1) * 128, s], in_=yt)

    nc.compile()
    return nc


def _make_in_maps(inputs):
    bf = ml_dtypes.bfloat16
    f8np = ml_dtypes.float8_e4m3
    x = np.asarray(inputs["x"], dtype=np.float32)
    cond = np.asarray(inputs["cond"], dtype=np.float32)
    ei = np.asarray(inputs["edge_index"])
    w_qkv = np.asarray(inputs["w_qkv"], dtype=np.float32)
    et = np.asarray(inputs["edge_table"], dtype=np.float32)

    scale = 1.0 / np.sqrt(HD)
    wqk = w_qkv[:, :2 * D].copy()
    wqk[:, :D] *= scale
    wv = np.ascontiguousarray(w_qkv[:, 2 * D:])
    wada = np.concatenate([inputs["w_ada1"], inputs["w_ada2"]],
                          axis=1).astype(np.float32)
    badar = np.concatenate([inputs["b_ada1"], inputs["b_ada2"]]).astype(
        np.float32).copy()
    badar[:D] += 1.0          # fold the (1 + scale) into ada1 scale bias
    badar[2 * D:3 * D] += 1.0  # and ada2 scale bias

    etT = np.exp(et).T.astype(np.float32)  # [H, 4]

    shared = {
        "wqk": np.ascontiguousarray(wqk.astype(bf)),
        "wv": wv.astype(bf),
        "wada": np.ascontiguousarray(wada.astype(bf)),
        "badar": np.ascontiguousarray(badar.reshape(1, 2 * 2 * D)),
        "wproj": np.ascontiguousarray(
            inputs["w_proj"].astype(np.float32).astype(bf)),
        "bproj": np.ascontiguousarray(
            inputs["b_proj"].astype(np.float32).reshape(4, 128)),
        "wm1": np.ascontiguousarray(
            inputs["w_mlp1"].astype(np.float32).astype(bf)),
        "bm1": np.ascontiguousarray(
            inputs["b_mlp1"].astype(np.float32).reshape(16, 128)),
        "wm2": np.ascontiguousarray(
            inputs["w_mlp2"].astype(np.float32).astype(bf)),
        "bm2": np.ascontiguousarray(
            inputs["b_mlp2"].astype(np.float32).reshape(4, 128)),
        "onesd": np.ones((128, 1), dtype=np.float32),
        "bandd": np.ascontiguousarray(np.concatenate([
            np.concatenate([np.ones(64), np.zeros(64)]),
            np.concatenate([np.zeros(64), np.ones(64)]),
        ]).reshape(2, 128).astype(bf)),
        "onesrowd": np.ones((1, 1024), dtype=np.float32),
    }
    in_maps = []
    for b in range(B):
        # g[h, j, i] = exp(et[ei[i, j], h])
        g = etT[:, ei[b]]                      # [H, i, j]
        g = np.ascontiguousarray(g.transpose(0, 2, 1))  # [H, j, i]
        in_maps.append(dict(
            shared,
            xT=np.ascontiguousarray(x[b].T),
            condc=np.ascontiguousarray(cond[b].reshape(4, 128)),
            gbias=g.reshape(H * V, V).astype(bf),
        ))
    return in_maps


def kernel(**inputs):
    from concourse.bass_utils import run_bass_kernel_spmd

    if "prog" not in _cache:
        _cache["prog"] = _build_program()
    nc = _cache["prog"]

    in_maps = _make_in_maps(inputs)
    res = run_bass_kernel_spmd(nc, in_maps, core_ids=list(range(NCORES)))
    out = np.stack([np.ascontiguousarray(res.results[b]["yT"].T)
                    for b in range(B)])
    return out.astype(np.float32)
